# revision 5
# baseline (speedup 1.0000x reference)
"""Trainium2 Bass kernel v9: per-superpixel mean of CNN features + linear head.

v4 structure (from v2/v3 traces):
  * features bf16, host-transposed to [f, pix]; proj = 2 stationary-ft
    matmuls per 128-pix tile into a shared PSUM bank (4 tiles per bank).
  * ONE broadcast tensor_tensor per 4-tile group reads the PSUM bank and
    builds the lo-expanded bf16 stationaries pq4 (s = 4*hi + lo).
  * the hi-onehot [128, 256] is precomputed on the HOST in fp8 (0/1 exact)
    and streamed from HBM with the features — v2/v3 traces showed the
    on-device is_equal at 371-2528 ns/tile on DVE; DMA has headroom
    (+8.4 MB/core on a 358 GB/s link) while DVE does not.
  * segsum: one matmul per tile, bf16 stationary x fp8 moving, fp32 PSUM
    accumulation across all 256 tiles.
"""

import os as _os

import numpy as np
import ml_dtypes

import concourse.mybir as mybir
import concourse.tile as tile
from concourse import bacc
from concourse.bass_utils import run_bass_kernel_spmd

N_CORES = 8
P = 128
F = 256
NUM_SP = 1024
C = 21
NPIX = 512 * 512
PIX_PER_CORE = NPIX // N_CORES            # 32768

FACTOR = 4
SHI = NUM_SP // FACTOR                    # 256
ST_COLS = C * FACTOR                      # 84

CHUNK_PIX = int(_os.environ.get("KV4_CHUNK_PIX", "4096"))
N_CHUNKS = PIX_PER_CORE // CHUNK_PIX      # 8
TILES_PER_CHUNK = CHUNK_PIX // P          # 32
N_TILES = PIX_PER_CORE // P               # 256

GROUP = int(_os.environ.get("KV4_GROUP", "4"))
GROUPS_PER_CHUNK = TILES_PER_CHUNK // GROUP
SWPG = int(_os.environ.get("KV4_SWPG", "3"))      # segsum delay in groups
CHUNK_BUFS = int(_os.environ.get("KV4_CHUNK_BUFS", "4"))
PSUM_BUFS = int(_os.environ.get("KV4_PSUM_BUFS", "5"))
# per-chunk: which groups' hi-onehots are generated on-device (DVE is_equal)
# vs DMA'd from host. Chunk 0 leans on DVE (its DMA is the ramp critical
# path); the last chunk is all-host (so DVE work drains before the tail).
def _dve_groups(c):
    if c == 0:
        return (1, 2, 5, 7)
    if c == N_CHUNKS - 1:
        return ()
    return (2, 4, 5, 7)


def _host_groups(c):
    return tuple(g for g in range(GROUPS_PER_CHUNK) if g not in _dve_groups(c))


N_HOST_G_MAX = max(len(_host_groups(c)) for c in range(N_CHUNKS))
WARM_MM = int(_os.environ.get("KV6_WARM_MM", "30"))
I16 = mybir.dt.int16

F32 = mybir.dt.float32
BF16 = mybir.dt.bfloat16
FP8 = mybir.dt.float8e4


def _build_nc():
    nc = bacc.Bacc("TRN2", target_bir_lowering=False)

    feats = nc.dram_tensor(
        "feats", [N_CHUNKS, 2, P, CHUNK_PIX], BF16, kind="ExternalInput"
    )
    ohhi = nc.dram_tensor(
        "ohhi", [N_CHUNKS, P, N_HOST_G_MAX * GROUP * SHI], FP8, kind="ExternalInput"
    )
    labels_hi = nc.dram_tensor("labels_hi", [P, N_TILES], mybir.dt.float32, kind="ExternalInput")
    iota = nc.dram_tensor("iota", [P, SHI], I16, kind="ExternalInput")
    oh_lo = nc.dram_tensor("oh_lo", [P, N_TILES * FACTOR], BF16, kind="ExternalInput")
    w_in = nc.dram_tensor("w_in", [P, 2 * C], BF16, kind="ExternalInput")
    out = nc.dram_tensor("out", [ST_COLS, SHI], F32, kind="ExternalOutput")

    with tile.TileContext(nc) as tc:
        with (
            tc.tile_pool(name="const", bufs=1) as const_pool,
            tc.tile_pool(name="chunk", bufs=CHUNK_BUFS) as chunk_pool,
            tc.tile_pool(name="pqpool", bufs=SWPG + 2) as pq_pool,
            tc.tile_pool(name="ohpool", bufs=GROUP * (SWPG + 4)) as oh_pool,
            tc.tile_pool(name="warmps", bufs=1, space="PSUM") as warm_pool,
            tc.tile_pool(name="psum", bufs=PSUM_BUFS, space="PSUM") as psum_pool,
            tc.tile_pool(name="accp", bufs=1, space="PSUM") as acc_pool,
        ):
            ohlo_sb = const_pool.tile([P, N_TILES * FACTOR], BF16)
            nc.sync.dma_start(out=ohlo_sb[:], in_=oh_lo[:])
            iota_sb = const_pool.tile([P, SHI], I16)
            nc.sync.dma_start(out=iota_sb[:], in_=iota[:])
            labels_sb = const_pool.tile([P, N_TILES], mybir.dt.float32)
            nc.sync.dma_start(out=labels_sb[:], in_=labels_hi[:])
            w_sb = const_pool.tile([P, 2 * C], BF16)
            nc.sync.dma_start(out=w_sb[:], in_=w_in[:])

            acc = acc_pool.tile([ST_COLS, SHI], F32)

            # HAM warmup: keep the PE busy from t=0 so the clock gate opens
            # (4096-cycle activity window) before the first chunk lands;
            # otherwise the first ~3.4us of real matmuls run at 1.2 GHz.
            if WARM_MM:
                warm_sb = const_pool.tile([P, P], BF16)
                nc.gpsimd.memset(warm_sb[:], 0.0)
                warm_ps = warm_pool.tile([P, P], F32)
                for _ in range(WARM_MM):
                    nc.tensor.matmul(
                        out=warm_ps[:],
                        lhsT=warm_sb[:],
                        rhs=warm_sb[:],
                        start=True,
                        stop=True,
                        skip_group_check=True,
                    )

            pending = []

            def emit_segsum(pq4g, rhss, tg0):
                for g in range(GROUP):
                    tg = tg0 + g
                    nc.tensor.matmul(
                        out=acc[:, :],
                        lhsT=pq4g[:, g * ST_COLS : (g + 1) * ST_COLS],
                        rhs=rhss[g],
                        start=tg < 1,
                        stop=tg >= N_TILES - 1,
                        skip_group_check=True,
                    )

            for c in range(N_CHUNKS):
                host_g = _host_groups(c)
                dve_g = _dve_groups(c)
                ohc_w = len(host_g) * GROUP * SHI
                feats_sb = chunk_pool.tile([P, 2 * CHUNK_PIX], BF16, tag="feats")
                ohc_sb = chunk_pool.tile([P, N_HOST_G_MAX * GROUP * SHI], FP8, tag="ohc")
                nsplit = 4 if c == 0 else (2 if c <= 2 else 1)
                q = CHUNK_PIX // nsplit
                qo = ohc_w // nsplit
                for k in range(nsplit):
                    for h in range(2):
                        nc.sync.dma_start(
                            out=feats_sb[:, h * CHUNK_PIX + k * q : h * CHUNK_PIX + (k + 1) * q],
                            in_=feats[c, h][:, k * q : (k + 1) * q],
                        )
                    if qo:
                        nc.sync.dma_start(
                            out=ohc_sb[:, k * qo : (k + 1) * qo],
                            in_=ohhi[c][:, k * qo : (k + 1) * qo],
                        )

                for grp in range(GROUPS_PER_CHUNK):
                    tg0 = c * TILES_PER_CHUNK + grp * GROUP
                    if grp in dve_g:
                        rhss = []
                        for g in range(GROUP):
                            tg = tg0 + g
                            oh = oh_pool.tile([P, SHI], BF16, tag="oh")
                            nc.vector.tensor_scalar(
                                oh[:],
                                iota_sb[:],
                                labels_sb[:, tg : tg + 1],
                                None,
                                mybir.AluOpType.is_equal,
                            )
                            rhss.append(oh[:])
                    else:
                        s = host_g.index(grp)
                        rhss = [
                            ohc_sb[
                                :, (s * GROUP + g) * SHI : (s * GROUP + g + 1) * SHI
                            ]
                            for g in range(GROUP)
                        ]

                    proj_ps = psum_pool.tile(
                        [P, GROUP * C], F32, tag="projps", padded_shape=[P, 512]
                    )
                    for g in range(GROUP):
                        t = grp * GROUP + g
                        for h in range(2):
                            nc.tensor.matmul(
                                out=proj_ps[:, g * C : (g + 1) * C],
                                lhsT=feats_sb[
                                    :, h * CHUNK_PIX + t * P : h * CHUNK_PIX + (t + 1) * P
                                ],
                                rhs=w_sb[:, h * C : (h + 1) * C],
                                start=h == 0,
                                stop=h == 1,
                                skip_group_check=True,
                            )

                    # one DVE op: pq4[p, g, j, c] = proj[p, g, c] * ohlo[p, g, j]
                    pq4g = pq_pool.tile([P, GROUP * ST_COLS], BF16, tag="pq4g")
                    nc.vector.tensor_tensor(
                        out=pq4g[:].rearrange("p (g j c) -> p g j c", g=GROUP, j=FACTOR),
                        in0=proj_ps[:]
                        .rearrange("p (g c) -> p g c", g=GROUP)[:, :, None, :]
                        .broadcast_to([P, GROUP, FACTOR, C]),
                        in1=ohlo_sb[:, tg0 * FACTOR : (tg0 + GROUP) * FACTOR]
                        .rearrange("p (g j) -> p g j", g=GROUP)[:, :, :, None]
                        .broadcast_to([P, GROUP, FACTOR, C]),
                        op=mybir.AluOpType.mult,
                    )

                    pending.append((pq4g, rhss, tg0))
                    if len(pending) > SWPG:
                        emit_segsum(*pending.pop(0))

            while pending:
                emit_segsum(*pending.pop(0))

            out_sb = chunk_pool.tile([ST_COLS, SHI], F32, tag="outsb")
            nc.scalar.activation(
                out=out_sb[:],
                in_=acc[:],
                func=mybir.ActivationFunctionType.Copy,
            )
            nc.sync.dma_start(out=out[:], in_=out_sb[:])

    nc.compile()
    return nc


def _install_ntff_hook():
    import contextlib
    import ctypes
    import sys
    import types

    if "antenv.axon_hooks" in sys.modules:
        return
    lib = ctypes.CDLL("/opt/axon/libaxon_pjrt.so")
    if not hasattr(lib, "axon_start_nrt_profile"):
        return
    lib.axon_start_nrt_profile.argtypes = [
        ctypes.POINTER(ctypes.c_int64),
        ctypes.c_size_t,
    ]
    lib.axon_start_nrt_profile.restype = ctypes.c_int64
    lib.axon_stop_nrt_profile.argtypes = [ctypes.c_char_p]
    lib.axon_stop_nrt_profile.restype = ctypes.c_int64

    @contextlib.contextmanager
    def _hook(output_dir, device_ids):
        import jax

        jax.devices()
        if device_ids:
            ids = (ctypes.c_int64 * len(device_ids))(*device_ids)
            rc = lib.axon_start_nrt_profile(ids, len(device_ids))
        else:
            rc = lib.axon_start_nrt_profile(None, 0)
        if rc != 0:
            raise RuntimeError(f"axon_start_nrt_profile rc={rc}")
        try:
            yield
        finally:
            n = lib.axon_stop_nrt_profile(str(output_dir).encode())
            print(f"profile: {n} file(s) written to {output_dir}", file=sys.stderr)

    mod = types.ModuleType("antenv.axon_hooks")
    mod.get_axon_ntff_profile_hook = lambda: _hook
    mod.set_axon_ntff_profile_hook = lambda h: None
    sys.modules["antenv.axon_hooks"] = mod


_NC_CACHE = None


def _get_nc():
    global _NC_CACHE
    if _NC_CACHE is None:
        _NC_CACHE = _build_nc()
    return _NC_CACHE


def kernel(features, superpixel, w_node):
    features = np.asarray(features, dtype=np.float32)
    superpixel = np.asarray(superpixel)
    w_node = np.asarray(w_node, dtype=np.float32)

    feats_flat = features.reshape(NPIX, F).astype(ml_dtypes.bfloat16)
    sp_flat = superpixel.reshape(NPIX).astype(np.int64)

    w_in = np.ascontiguousarray(
        w_node.T.reshape(2, P, C).transpose(1, 0, 2).reshape(P, 2 * C)
    ).astype(ml_dtypes.bfloat16)

    shi_ar = np.arange(SHI, dtype=np.int64)
    fac_ar = np.arange(FACTOR, dtype=np.int64)
    in_maps = []
    for core in range(N_CORES):
        lo = core * PIX_PER_CORE
        fc = feats_flat[lo : lo + PIX_PER_CORE]
        spc = sp_flat[lo : lo + PIX_PER_CORE]
        fT = np.ascontiguousarray(
            fc.reshape(N_CHUNKS, CHUNK_PIX, 2, P).transpose(0, 2, 3, 1)
        )
        # pixel index within core = c*CHUNK + t*128 + p ; tg = c*TPC + t
        lab = spc.reshape(N_CHUNKS, TILES_PER_CHUNK, P).transpose(2, 0, 1).reshape(
            P, N_TILES
        )
        oh_lo = (
            (lab[:, :, None] % FACTOR) == fac_ar[None, None, :]
        ).reshape(P, N_TILES * FACTOR).astype(ml_dtypes.bfloat16)
        # ohhi packs only each chunk's host groups: [c, p, (s*GROUP+g)*SHI+h]
        labhi = (lab // FACTOR).reshape(P, N_CHUNKS, GROUPS_PER_CHUNK, GROUP)
        ohhi = np.zeros(
            (N_CHUNKS, P, N_HOST_G_MAX * GROUP * SHI), dtype=ml_dtypes.float8_e4m3
        )
        for c in range(N_CHUNKS):
            hg = list(_host_groups(c))
            oh_c = (
                (labhi[:, c, hg, :, None] == shi_ar[None, None, None, :])
                .reshape(P, len(hg) * GROUP * SHI)
                .astype(ml_dtypes.float8_e4m3)
            )
            ohhi[c, :, : oh_c.shape[1]] = oh_c
        in_maps.append(
            {
                "feats": fT,
                "ohhi": np.ascontiguousarray(ohhi),
                "oh_lo": np.ascontiguousarray(oh_lo),
                "labels_hi": (lab // FACTOR).astype(np.float32),
                "iota": np.broadcast_to(
                    np.arange(SHI, dtype=np.int16)[None, :], (P, SHI)
                ).copy(),
                "w_in": w_in,
            }
        )

    trace = bool(int(_os.environ.get("KERNEL_TRACE", "0")))
    repeat = int(_os.environ.get("KERNEL_REPEAT", "1"))
    kwargs = {}
    if trace:
        _install_ntff_hook()
        import concourse.bass_utils as _bu

        _bu.upload_artifacts = lambda tmpdir: tmpdir
    base_dir = _os.environ.get("KERNEL_TRACE_DIR") or None
    for rep in range(repeat):
        if trace and base_dir:
            kwargs["tmpdir"] = _os.path.join(base_dir, f"rep{rep}")
            _os.makedirs(kwargs["tmpdir"], exist_ok=True)
        res = run_bass_kernel_spmd(
            _get_nc(), in_maps, core_ids=list(range(N_CORES)), trace=trace, **kwargs
        )
        if trace:
            print(f"HW exec time: {res.exec_time_ns} ns")
            print(f"profile_json: {res.profile_json}")

    total = np.zeros((NUM_SP, C), dtype=np.float64)
    for r in res.results:
        o = np.asarray(r["out"], dtype=np.float64)       # [ST_COLS, SHI]
        o = o.reshape(FACTOR, C, SHI).transpose(2, 0, 1).reshape(NUM_SP, C)
        total += o
    counts = np.bincount(sp_flat, minlength=NUM_SP).astype(np.float64)
    node_potentials = total / np.clip(counts, 1.0, None)[:, None]
    return np.ascontiguousarray(node_potentials).astype(np.float32)


# revision 6
# speedup vs baseline: 1.1786x; 1.1786x over previous
"""Trainium2 Bass kernel v9: per-superpixel mean of CNN features + linear head.

v4 structure (from v2/v3 traces):
  * features bf16, host-transposed to [f, pix]; proj = 2 stationary-ft
    matmuls per 128-pix tile into a shared PSUM bank (4 tiles per bank).
  * ONE broadcast tensor_tensor per 4-tile group reads the PSUM bank and
    builds the lo-expanded bf16 stationaries pq4 (s = 4*hi + lo).
  * the hi-onehot [128, 256] is precomputed on the HOST in fp8 (0/1 exact)
    and streamed from HBM with the features — v2/v3 traces showed the
    on-device is_equal at 371-2528 ns/tile on DVE; DMA has headroom
    (+8.4 MB/core on a 358 GB/s link) while DVE does not.
  * segsum: one matmul per tile, bf16 stationary x fp8 moving, fp32 PSUM
    accumulation across all 256 tiles.
"""

import os as _os

import numpy as np
import ml_dtypes

import concourse.mybir as mybir
import concourse.tile as tile
from concourse import bacc
from concourse.bass_utils import run_bass_kernel_spmd

N_CORES = 8
P = 128
F = 256
NUM_SP = 1024
C = 21
NPIX = 512 * 512
PIX_PER_CORE = NPIX // N_CORES            # 32768

FACTOR = 4
SHI = NUM_SP // FACTOR                    # 256
ST_COLS = C * FACTOR                      # 84

CHUNK_PIX = int(_os.environ.get("KV4_CHUNK_PIX", "4096"))
N_CHUNKS = PIX_PER_CORE // CHUNK_PIX      # 8
TILES_PER_CHUNK = CHUNK_PIX // P          # 32
N_TILES = PIX_PER_CORE // P               # 256

GROUP = int(_os.environ.get("KV4_GROUP", "4"))
GROUPS_PER_CHUNK = TILES_PER_CHUNK // GROUP
SWPG = int(_os.environ.get("KV4_SWPG", "3"))      # segsum delay in groups
CHUNK_BUFS = int(_os.environ.get("KV4_CHUNK_BUFS", "4"))
PSUM_BUFS = int(_os.environ.get("KV4_PSUM_BUFS", "5"))
# per-chunk: which groups' hi-onehots are generated on-device (DVE is_equal)
# vs DMA'd from host. Chunk 0 leans on DVE (its DMA is the ramp critical
# path); the last chunk is all-host (so DVE work drains before the tail).
def _dve_groups(c):
    if c == 0:
        return (1, 2, 5, 7)
    if c == N_CHUNKS - 1:
        return ()
    return (2, 4, 5, 7)


def _host_groups(c):
    return tuple(g for g in range(GROUPS_PER_CHUNK) if g not in _dve_groups(c))


N_HOST_G_MAX = max(len(_host_groups(c)) for c in range(N_CHUNKS))
WARM_MM = int(_os.environ.get("KV6_WARM_MM", "60"))
I16 = mybir.dt.int16

F32 = mybir.dt.float32
BF16 = mybir.dt.bfloat16
FP8 = mybir.dt.float8e4


def _build_nc():
    nc = bacc.Bacc("TRN2", target_bir_lowering=False)

    feats = nc.dram_tensor(
        "feats", [N_CHUNKS, 2, P, CHUNK_PIX], BF16, kind="ExternalInput"
    )
    ohhi = nc.dram_tensor(
        "ohhi", [N_CHUNKS, P, N_HOST_G_MAX * GROUP * SHI], FP8, kind="ExternalInput"
    )
    labels_hi = nc.dram_tensor("labels_hi", [P, N_TILES], mybir.dt.float32, kind="ExternalInput")
    iota = nc.dram_tensor("iota", [P, SHI], I16, kind="ExternalInput")
    oh_lo = nc.dram_tensor("oh_lo", [P, N_TILES * FACTOR], BF16, kind="ExternalInput")
    w_in = nc.dram_tensor("w_in", [P, 2 * C], BF16, kind="ExternalInput")
    out = nc.dram_tensor("out", [ST_COLS, SHI], F32, kind="ExternalOutput")

    with tile.TileContext(nc) as tc:
        with (
            tc.tile_pool(name="const", bufs=1) as const_pool,
            tc.tile_pool(name="chunk", bufs=CHUNK_BUFS) as chunk_pool,
            tc.tile_pool(name="pqpool", bufs=SWPG + 2) as pq_pool,
            tc.tile_pool(name="ohpool", bufs=GROUP * (SWPG + 4)) as oh_pool,
            tc.tile_pool(name="warmps", bufs=1, space="PSUM") as warm_pool,
            tc.tile_pool(name="psum", bufs=PSUM_BUFS, space="PSUM") as psum_pool,
            tc.tile_pool(name="accp", bufs=1, space="PSUM") as acc_pool,
        ):
            ohlo_sb = const_pool.tile([P, N_TILES * FACTOR], BF16)
            nc.sync.dma_start(out=ohlo_sb[:], in_=oh_lo[:])
            iota_sb = const_pool.tile([P, SHI], I16)
            nc.sync.dma_start(out=iota_sb[:], in_=iota[:])
            labels_sb = const_pool.tile([P, N_TILES], mybir.dt.float32)
            nc.sync.dma_start(out=labels_sb[:], in_=labels_hi[:])
            w_sb = const_pool.tile([P, 2 * C], BF16)
            nc.sync.dma_start(out=w_sb[:], in_=w_in[:])

            acc = acc_pool.tile([ST_COLS, SHI], F32)

            # HAM warmup: keep the PE busy from t=0 so the clock gate opens
            # (4096-cycle activity window) before the first chunk lands;
            # otherwise the first ~3.4us of real matmuls run at 1.2 GHz.
            if WARM_MM:
                warm_sb = const_pool.tile([P, P], BF16)
                nc.gpsimd.memset(warm_sb[:], 0.0)
                warm_ps = warm_pool.tile([P, P], F32)
                for _ in range(WARM_MM):
                    nc.tensor.matmul(
                        out=warm_ps[:],
                        lhsT=warm_sb[:],
                        rhs=warm_sb[:],
                        start=True,
                        stop=True,
                        skip_group_check=True,
                    )

            pending = []

            def emit_segsum(pq4g, rhss, tg0):
                for g in range(GROUP):
                    tg = tg0 + g
                    nc.tensor.matmul(
                        out=acc[:, :],
                        lhsT=pq4g[:, g * ST_COLS : (g + 1) * ST_COLS],
                        rhs=rhss[g],
                        start=tg < 1,
                        stop=tg >= N_TILES - 1,
                        skip_group_check=True,
                    )

            for c in range(N_CHUNKS):
                host_g = _host_groups(c)
                dve_g = _dve_groups(c)
                ohc_w = len(host_g) * GROUP * SHI
                feats_sb = chunk_pool.tile([P, 2 * CHUNK_PIX], BF16, tag="feats")
                ohc_sb = chunk_pool.tile([P, N_HOST_G_MAX * GROUP * SHI], FP8, tag="ohc")
                nsplit = 4 if c == 0 else (2 if c <= 2 else 1)
                q = CHUNK_PIX // nsplit
                qo = ohc_w // nsplit
                for k in range(nsplit):
                    for h in range(2):
                        nc.sync.dma_start(
                            out=feats_sb[:, h * CHUNK_PIX + k * q : h * CHUNK_PIX + (k + 1) * q],
                            in_=feats[c, h][:, k * q : (k + 1) * q],
                        )
                    if qo:
                        nc.sync.dma_start(
                            out=ohc_sb[:, k * qo : (k + 1) * qo],
                            in_=ohhi[c][:, k * qo : (k + 1) * qo],
                        )

                for grp in range(GROUPS_PER_CHUNK):
                    tg0 = c * TILES_PER_CHUNK + grp * GROUP
                    if grp in dve_g:
                        rhss = []
                        for g in range(GROUP):
                            tg = tg0 + g
                            oh = oh_pool.tile([P, SHI], BF16, tag="oh")
                            nc.vector.tensor_scalar(
                                oh[:],
                                iota_sb[:],
                                labels_sb[:, tg : tg + 1],
                                None,
                                mybir.AluOpType.is_equal,
                            )
                            rhss.append(oh[:])
                    else:
                        s = host_g.index(grp)
                        rhss = [
                            ohc_sb[
                                :, (s * GROUP + g) * SHI : (s * GROUP + g + 1) * SHI
                            ]
                            for g in range(GROUP)
                        ]

                    proj_ps = psum_pool.tile(
                        [P, GROUP * C], F32, tag="projps", padded_shape=[P, 512]
                    )
                    for g in range(GROUP):
                        t = grp * GROUP + g
                        for h in range(2):
                            nc.tensor.matmul(
                                out=proj_ps[:, g * C : (g + 1) * C],
                                lhsT=feats_sb[
                                    :, h * CHUNK_PIX + t * P : h * CHUNK_PIX + (t + 1) * P
                                ],
                                rhs=w_sb[:, h * C : (h + 1) * C],
                                start=h == 0,
                                stop=h == 1,
                                skip_group_check=True,
                            )

                    # one DVE op: pq4[p, g, j, c] = proj[p, g, c] * ohlo[p, g, j]
                    pq4g = pq_pool.tile([P, GROUP * ST_COLS], BF16, tag="pq4g")
                    nc.vector.tensor_tensor(
                        out=pq4g[:].rearrange("p (g j c) -> p g j c", g=GROUP, j=FACTOR),
                        in0=proj_ps[:]
                        .rearrange("p (g c) -> p g c", g=GROUP)[:, :, None, :]
                        .broadcast_to([P, GROUP, FACTOR, C]),
                        in1=ohlo_sb[:, tg0 * FACTOR : (tg0 + GROUP) * FACTOR]
                        .rearrange("p (g j) -> p g j", g=GROUP)[:, :, :, None]
                        .broadcast_to([P, GROUP, FACTOR, C]),
                        op=mybir.AluOpType.mult,
                    )

                    pending.append((pq4g, rhss, tg0))
                    if len(pending) > SWPG:
                        emit_segsum(*pending.pop(0))

            while pending:
                emit_segsum(*pending.pop(0))

            out_sb = chunk_pool.tile([ST_COLS, SHI], F32, tag="outsb")
            nc.scalar.activation(
                out=out_sb[:],
                in_=acc[:],
                func=mybir.ActivationFunctionType.Copy,
            )
            nc.sync.dma_start(out=out[:], in_=out_sb[:])

    nc.compile()
    return nc


def _install_ntff_hook():
    import contextlib
    import ctypes
    import sys
    import types

    if "antenv.axon_hooks" in sys.modules:
        return
    lib = ctypes.CDLL("/opt/axon/libaxon_pjrt.so")
    if not hasattr(lib, "axon_start_nrt_profile"):
        return
    lib.axon_start_nrt_profile.argtypes = [
        ctypes.POINTER(ctypes.c_int64),
        ctypes.c_size_t,
    ]
    lib.axon_start_nrt_profile.restype = ctypes.c_int64
    lib.axon_stop_nrt_profile.argtypes = [ctypes.c_char_p]
    lib.axon_stop_nrt_profile.restype = ctypes.c_int64

    @contextlib.contextmanager
    def _hook(output_dir, device_ids):
        import jax

        jax.devices()
        if device_ids:
            ids = (ctypes.c_int64 * len(device_ids))(*device_ids)
            rc = lib.axon_start_nrt_profile(ids, len(device_ids))
        else:
            rc = lib.axon_start_nrt_profile(None, 0)
        if rc != 0:
            raise RuntimeError(f"axon_start_nrt_profile rc={rc}")
        try:
            yield
        finally:
            n = lib.axon_stop_nrt_profile(str(output_dir).encode())
            print(f"profile: {n} file(s) written to {output_dir}", file=sys.stderr)

    mod = types.ModuleType("antenv.axon_hooks")
    mod.get_axon_ntff_profile_hook = lambda: _hook
    mod.set_axon_ntff_profile_hook = lambda h: None
    sys.modules["antenv.axon_hooks"] = mod


_NC_CACHE = None


def _get_nc():
    global _NC_CACHE
    if _NC_CACHE is None:
        _NC_CACHE = _build_nc()
    return _NC_CACHE


def kernel(features, superpixel, w_node):
    features = np.asarray(features, dtype=np.float32)
    superpixel = np.asarray(superpixel)
    w_node = np.asarray(w_node, dtype=np.float32)

    feats_flat = features.reshape(NPIX, F).astype(ml_dtypes.bfloat16)
    sp_flat = superpixel.reshape(NPIX).astype(np.int64)

    w_in = np.ascontiguousarray(
        w_node.T.reshape(2, P, C).transpose(1, 0, 2).reshape(P, 2 * C)
    ).astype(ml_dtypes.bfloat16)

    shi_ar = np.arange(SHI, dtype=np.int64)
    fac_ar = np.arange(FACTOR, dtype=np.int64)
    in_maps = []
    for core in range(N_CORES):
        lo = core * PIX_PER_CORE
        fc = feats_flat[lo : lo + PIX_PER_CORE]
        spc = sp_flat[lo : lo + PIX_PER_CORE]
        fT = np.ascontiguousarray(
            fc.reshape(N_CHUNKS, CHUNK_PIX, 2, P).transpose(0, 2, 3, 1)
        )
        # pixel index within core = c*CHUNK + t*128 + p ; tg = c*TPC + t
        lab = spc.reshape(N_CHUNKS, TILES_PER_CHUNK, P).transpose(2, 0, 1).reshape(
            P, N_TILES
        )
        oh_lo = (
            (lab[:, :, None] % FACTOR) == fac_ar[None, None, :]
        ).reshape(P, N_TILES * FACTOR).astype(ml_dtypes.bfloat16)
        # ohhi packs only each chunk's host groups: [c, p, (s*GROUP+g)*SHI+h]
        labhi = (lab // FACTOR).reshape(P, N_CHUNKS, GROUPS_PER_CHUNK, GROUP)
        ohhi = np.zeros(
            (N_CHUNKS, P, N_HOST_G_MAX * GROUP * SHI), dtype=ml_dtypes.float8_e4m3
        )
        for c in range(N_CHUNKS):
            hg = list(_host_groups(c))
            oh_c = (
                (labhi[:, c, hg, :, None] == shi_ar[None, None, None, :])
                .reshape(P, len(hg) * GROUP * SHI)
                .astype(ml_dtypes.float8_e4m3)
            )
            ohhi[c, :, : oh_c.shape[1]] = oh_c
        in_maps.append(
            {
                "feats": fT,
                "ohhi": np.ascontiguousarray(ohhi),
                "oh_lo": np.ascontiguousarray(oh_lo),
                "labels_hi": (lab // FACTOR).astype(np.float32),
                "iota": np.broadcast_to(
                    np.arange(SHI, dtype=np.int16)[None, :], (P, SHI)
                ).copy(),
                "w_in": w_in,
            }
        )

    trace = bool(int(_os.environ.get("KERNEL_TRACE", "0")))
    repeat = int(_os.environ.get("KERNEL_REPEAT", "1"))
    kwargs = {}
    if trace:
        _install_ntff_hook()
        import concourse.bass_utils as _bu

        _bu.upload_artifacts = lambda tmpdir: tmpdir
    base_dir = _os.environ.get("KERNEL_TRACE_DIR") or None
    for rep in range(repeat):
        if trace and base_dir:
            kwargs["tmpdir"] = _os.path.join(base_dir, f"rep{rep}")
            _os.makedirs(kwargs["tmpdir"], exist_ok=True)
        res = run_bass_kernel_spmd(
            _get_nc(), in_maps, core_ids=list(range(N_CORES)), trace=trace, **kwargs
        )
        if trace:
            print(f"HW exec time: {res.exec_time_ns} ns")
            print(f"profile_json: {res.profile_json}")

    total = np.zeros((NUM_SP, C), dtype=np.float64)
    for r in res.results:
        o = np.asarray(r["out"], dtype=np.float64)       # [ST_COLS, SHI]
        o = o.reshape(FACTOR, C, SHI).transpose(2, 0, 1).reshape(NUM_SP, C)
        total += o
    counts = np.bincount(sp_flat, minlength=NUM_SP).astype(np.float64)
    node_potentials = total / np.clip(counts, 1.0, None)[:, None]
    return np.ascontiguousarray(node_potentials).astype(np.float32)
